# revision 15
# baseline (speedup 1.0000x reference)
"""Expert-choice router kernel for Trainium2 (8 NeuronCores, Bass/Tile).

Strategy (data-parallel over batch, expert-parallel for selection):
  Phase A (per core, 2 batches):  logits^T = W @ x^T via PE (x transposed
          on-chip with PE transpose-mode), fused exp on ACT during PSUM
          evacuation, softmax over the sequence axis in [E, S] layout.
  Phase B: AllGather of per-core scores^T (256 KB each) so every core can
          see all tokens for its 2 experts.
  Phase C (per core, 2 experts): exact per-expert threshold = value t with
          count(score >= t) == capacity, found by branchless bisection /
          regula-falsi iterations on DVE with PE matmuls doing the
          cross-partition count reduction + threshold broadcast.
  Phase D: tiny AllGather of the 16 thresholds; per-token apply phase
          computes argmax-selected expert + routing weight with a
          pack-into-one-float trick and partition-folded maxes.
"""

import os
import sys

for _p in ("/opt/trn_rl_repo", "/root/.axon_site/_ro/trn_rl_repo"):
    if os.path.isdir(_p) and _p not in sys.path:
        sys.path.insert(0, _p)

import numpy as np


def _install_ntff_hook_module():
    """Provide antenv.axon_hooks (missing in this image) via sys.modules so
    run_bass_kernel_spmd(trace=True) can drive NTFF profiling through
    libaxon_pjrt.so. No-op when already present."""
    import types
    import importlib

    try:
        importlib.import_module("antenv.axon_hooks")
        return
    except ImportError:
        pass
    import contextlib
    import ctypes

    mod = types.ModuleType("antenv.axon_hooks")
    _state = {"hook": None}

    def set_axon_ntff_profile_hook(hook):
        _state["hook"] = hook

    def _via_ctypes(so_path):
        lib = ctypes.CDLL(so_path)
        if not hasattr(lib, "axon_start_nrt_profile"):
            return None
        lib.axon_start_nrt_profile.argtypes = [
            ctypes.POINTER(ctypes.c_int64),
            ctypes.c_size_t,
        ]
        lib.axon_start_nrt_profile.restype = ctypes.c_int64
        lib.axon_stop_nrt_profile.argtypes = [ctypes.c_char_p]
        lib.axon_stop_nrt_profile.restype = ctypes.c_int64

        @contextlib.contextmanager
        def _hook(output_dir, device_ids):
            import jax

            jax.devices()
            if device_ids:
                ids = (ctypes.c_int64 * len(device_ids))(*device_ids)
                rc = lib.axon_start_nrt_profile(ids, len(device_ids))
            else:
                rc = lib.axon_start_nrt_profile(None, 0)
            if rc != 0:
                raise RuntimeError(f"axon_start_nrt_profile rc={rc}")
            try:
                yield
            finally:
                n = lib.axon_stop_nrt_profile(str(output_dir).encode())
                print(f"ntff profile: {n} file(s) -> {output_dir}", file=sys.stderr)

        return _hook

    def get_axon_ntff_profile_hook():
        if _state["hook"] is None and os.path.exists("/opt/axon/libaxon_pjrt.so"):
            _state["hook"] = _via_ctypes("/opt/axon/libaxon_pjrt.so")
        return _state["hook"]

    mod.set_axon_ntff_profile_hook = set_axon_ntff_profile_hook
    mod.get_axon_ntff_profile_hook = get_axon_ntff_profile_hook
    sys.modules["antenv.axon_hooks"] = mod


_install_ntff_hook_module()

import concourse.bass as bass
import concourse.bacc as bacc
import concourse.mybir as mybir
import concourse.tile as tile
from concourse import bass_utils

Alu = mybir.AluOpType
Act = mybir.ActivationFunctionType
F32 = mybir.dt.float32
I32 = mybir.dt.int32
AX = mybir.AxisListType

B, S, D, E = 16, 2048, 1024, 16
NCORES = 8
BL = B // NCORES            # batches per core
SL = BL * S                 # tokens per core (4096)
N = B * S
CAP = int(1.25 * N / E)     # 2560
EL = E // NCORES            # experts per core in selection phase
P = 128

# Bisection schedule: fixed probes first (bracket the threshold region),
# then alternating bisection / regula-falsi. Tuned by host simulation with
# safety margin; each iteration is branchless so extra iterations are no-ops
# once count(>=T) == CAP has been observed (sticky `ans`).
PROBES = [1.0 / 2048, 2.0 / 2048, 4.0 / 2048, 8.0 / 2048]
ITERS = 26


def _build_program():
    nc = bacc.Bacc(
        "TRN2",
        target_bir_lowering=False,
        debug=False,
        num_devices=NCORES,
    )

    xs = nc.dram_tensor("xs", [SL, D], F32, kind="ExternalInput")
    wt = nc.dram_tensor("wt", [P, 8 * E], F32, kind="ExternalInput")
    ident = nc.dram_tensor("ident", [P, P], F32, kind="ExternalInput")
    gcst = nc.dram_tensor("gcst", [P, EL], F32, kind="ExternalInput")
    hcst = nc.dram_tensor("hcst", [EL, P], F32, kind="ExternalInput")
    rcst = nc.dram_tensor("rcst", [E, P], F32, kind="ExternalInput")
    ecst = nc.dram_tensor("ecst", [P, 1], F32, kind="ExternalInput")

    scores_out = nc.dram_tensor("scores_out", [SL, E], F32, kind="ExternalOutput")
    dbg_out = nc.dram_tensor("dbg_out", [EL, 64], F32, kind="ExternalOutput")
    rw_out = nc.dram_tensor("rw_out", [SL], F32, kind="ExternalOutput")
    ei_out = nc.dram_tensor("ei_out", [SL], I32, kind="ExternalOutput")

    sc_loc = nc.dram_tensor("sc_loc", [E, SL], F32, kind="Internal")
    sc_glob = nc.dram_tensor(
        "sc_glob", [NCORES, E, SL], F32, kind="Internal", addr_space="Shared"
    )
    thr_loc = nc.dram_tensor("thr_loc", [EL, 16], F32, kind="Internal")
    thr_glob = nc.dram_tensor(
        "thr_glob", [NCORES, EL, 16], F32, kind="Internal", addr_space="Shared"
    )

    with tile.TileContext(nc) as tc:
        _build_tile(tc, nc, locals())

    nc.compile()
    return nc


def _build_tile(tc, nc, t):
    xs, wt, ident = t["xs"], t["wt"], t["ident"]
    gcst, hcst, rcst, ecst = t["gcst"], t["hcst"], t["rcst"], t["ecst"]
    scores_out, rw_out, ei_out = t["scores_out"], t["rw_out"], t["ei_out"]
    dbg_out = t["dbg_out"]
    sc_loc, sc_glob, thr_loc, thr_glob = (
        t["sc_loc"], t["sc_glob"], t["thr_loc"], t["thr_glob"],
    )

    from contextlib import ExitStack

    ctx = ExitStack()
    consts = ctx.enter_context(tc.tile_pool(name="consts", bufs=1))
    xin = ctx.enter_context(tc.tile_pool(name="xin", bufs=3))
    xtp = ctx.enter_context(tc.tile_pool(name="xtp", bufs=3))
    scp = ctx.enter_context(tc.tile_pool(name="scp", bufs=1))
    misc = ctx.enter_context(tc.tile_pool(name="misc", bufs=2))
    psA = ctx.enter_context(tc.tile_pool(name="psA", bufs=2, space="PSUM"))
    psL = ctx.enter_context(tc.tile_pool(name="psL", bufs=2, space="PSUM"))
    psS = ctx.enter_context(tc.tile_pool(name="psS", bufs=2, space="PSUM"))

    # ---- constants ----
    wt_sb = consts.tile([P, 8 * E], F32)
    nc.sync.dma_start(wt_sb[:], wt.ap())
    id_sb = consts.tile([P, P], F32)
    nc.sync.dma_start(id_sb[:], ident.ap())
    g_sb = consts.tile([P, EL], F32)
    nc.sync.dma_start(g_sb[:], gcst.ap())
    h_sb = consts.tile([EL, P], F32)
    nc.sync.dma_start(h_sb[:], hcst.ap())
    r_sb = consts.tile([E, P], F32)
    nc.sync.dma_start(r_sb[:], rcst.ap())
    e_sb = consts.tile([P, 1], F32)
    nc.sync.dma_start(e_sb[:], ecst.ap())
    ones_sb = consts.tile([P, 512], F32)
    nc.vector.memset(ones_sb[:], 1.0)

    # ---- Phase A: logits^T + exp, per 512-token group ----
    scT = scp.tile([E, SL], F32)          # exp(logits)^T, later scores^T
    denp = consts.tile([E, 8], F32)       # per-group exp-sum partials
    xs_r = xs.ap().rearrange("(g t p) d -> g p t d", p=P, t=4)

    for g in range(8):
        x_sb = xin.tile([P, 4, D], F32)
        nc.sync.dma_start(x_sb[:], xs_r[g])
        lps = psL.tile([E, 512], F32)
        for k in range(8):
            tps = psA.tile([P, 512], F32, tag="tps")
            for tt in range(4):
                nc.tensor.transpose(
                    tps[:, tt * P:(tt + 1) * P],
                    x_sb[:, tt, k * P:(k + 1) * P],
                    id_sb[:],
                )
            xt_sb = xtp.tile([P, 512], F32, tag="xt")
            nc.vector.tensor_copy(xt_sb[:], tps[:])
            nc.tensor.matmul(
                lps[:],
                wt_sb[:, k * E:(k + 1) * E],
                xt_sb[:],
                start=(k == 0),
                stop=(k == 7),
            )
        # fused exp during PSUM evacuation + free row-sum partial
        nc.scalar.activation(
            scT[:, g * 512:(g + 1) * 512],
            lps[:],
            Act.Exp,
            accum_out=denp[:, g:g + 1],
        )

    # ---- softmax normalization over sequence axis (per local batch) ----
    den = misc.tile([E, BL], F32, tag="den")
    rden = misc.tile([E, BL], F32, tag="rden")
    for b in range(BL):
        nc.vector.tensor_reduce(
            den[:, b:b + 1], denp[:, b * 4:(b + 1) * 4], AX.X, Alu.add
        )
    nc.vector.reciprocal(rden[:], den[:])
    for b in range(BL):
        nc.vector.tensor_scalar(
            scT[:, b * S:(b + 1) * S],
            scT[:, b * S:(b + 1) * S],
            rden[:, b:b + 1],
            None,
            op0=Alu.mult,
        )

    # scores^T -> DRAM (collective input + apply-phase reload source)
    nc.sync.dma_start(sc_loc.ap(), scT[:])

    # ---- scores output in [s, e] orientation via PE transpose ----
    for tt in range(SL // P):
        sps = psS.tile([P, E], F32, tag="small")
        nc.tensor.transpose(sps[:], scT[:, tt * P:(tt + 1) * P], id_sb[:E, :E])
        so_sb = xtp.tile([P, E], F32, tag="so")
        nc.vector.tensor_copy(so_sb[:], sps[:])
        nc.sync.dma_start(scores_out.ap()[tt * P:(tt + 1) * P, :], so_sb[:])

    # ---- Phase B: exchange scores ----
    nc.gpsimd.collective_compute(
        "AllGather",
        Alu.bypass,
        replica_groups=[list(range(NCORES))],
        ins=[sc_loc.ap().opt()],
        outs=[sc_glob.ap().opt()],
    )

    # ---- Phase C: exact per-expert thresholds (2 experts per core) ----
    pid = nc.partition_id()
    v_sb = scp.tile([P, 512], F32)        # [2 experts x 64 chunks, 512]
    sgl = sc_glob.ap()                     # [8, 16, 4096]
    for ei in range(EL):
        src = sgl[:, bass.ds(EL * pid + ei, 1), :].rearrange(
            "c o (u f) -> c (o u) f", f=512
        )
        nc.sync.dma_start(v_sb[ei * 64:(ei + 1) * 64, :], src)

    lo = misc.tile([EL, 1], F32, tag="lo")
    hi = misc.tile([EL, 1], F32, tag="hi")
    clo = misc.tile([EL, 1], F32, tag="clo")
    chi = misc.tile([EL, 1], F32, tag="chi")
    ans = misc.tile([EL, 1], F32, tag="ans")
    Tp = misc.tile([EL, 1], F32, tag="Tp")
    U32 = mybir.dt.uint32
    dmask = misc.tile([EL, 1], U32, tag="dmask")
    dnmask = misc.tile([EL, 1], U32, tag="dnmask")
    eqm = misc.tile([EL, 1], U32, tag="eqm")
    sc1 = misc.tile([EL, 1], F32, tag="sc1")
    sc2 = misc.tile([EL, 1], F32, tag="sc2")
    sc3 = misc.tile([EL, 1], F32, tag="sc3")
    cnt_sb = misc.tile([EL, 1], F32, tag="cntsb")
    nc.vector.memset(lo[:], 0.0)
    nc.vector.memset(hi[:], 1.0)
    nc.vector.memset(clo[:], float(N))
    nc.vector.memset(chi[:], 0.0)
    nc.vector.memset(ans[:], 0.0)

    cpp = misc.tile([P, 1], F32, tag="cpp")
    scr = misc.tile([P, 512], F32, tag="scr")
    dbg = misc.tile([EL, 64], F32, tag="dbg")
    nc.vector.memset(dbg[:], 0.0)

    for it in range(ITERS):
        if it < len(PROBES):
            nc.vector.memset(Tp[:], float(PROBES[it]))
        elif it % 2 == 0:
            # bisection step: T = (lo + hi) / 2
            nc.vector.tensor_add(Tp[:], lo[:], hi[:])
            nc.vector.tensor_scalar_mul(Tp[:], Tp[:], 0.5)
        else:
            # regula falsi: T = lo + (clo-CAP)/(clo-chi) * (hi-lo)
            nc.vector.tensor_scalar(sc1[:], clo[:], float(CAP), None, op0=Alu.subtract)
            nc.vector.tensor_sub(sc2[:], clo[:], chi[:])
            nc.vector.reciprocal(sc3[:], sc2[:])
            nc.vector.tensor_mul(sc1[:], sc1[:], sc3[:])
            nc.vector.tensor_sub(sc2[:], hi[:], lo[:])
            nc.vector.tensor_mul(sc1[:], sc1[:], sc2[:])
            nc.vector.tensor_add(Tp[:], lo[:], sc1[:])

        # broadcast T to the 128 partitions: Tb[p] = Tp[p // 64]
        Tb = psS.tile([P, 1], F32, tag="small")
        nc.tensor.matmul(Tb[:], h_sb[:], Tp[:])
        # per-partition counts of (v >= T)
        for ei in range(EL):
            sl = slice(ei * 64, (ei + 1) * 64)
            nc.vector.scalar_tensor_tensor(
                scr[sl, :],
                v_sb[sl, :],
                Tb[sl, :],
                ones_sb[sl, :],
                op0=Alu.is_ge,
                op1=Alu.mult,
                accum_out=cpp[sl, :],
            )
        # total count per expert: cps[e] = sum over the expert's 64 partitions
        cps = psS.tile([EL, 1], F32, tag="small2")
        nc.tensor.matmul(cps[:], g_sb[:], cpp[:])

        nc.vector.tensor_copy(cnt_sb[:], cps[:])
        nc.vector.tensor_copy(dbg[:, it:it + 1], Tp[:])
        nc.vector.tensor_copy(dbg[:, 32 + it:33 + it], cnt_sb[:])
        # branchless bracket update; out must only alias on_false (select
        # lowers to copy(on_false) + copy_predicated(on_true))
        nc.vector.tensor_scalar(dmask[:], cnt_sb[:], float(CAP), None, op0=Alu.is_ge)
        nc.vector.tensor_scalar(dnmask[:], cnt_sb[:], float(CAP), None, op0=Alu.is_lt)
        nc.vector.tensor_scalar(eqm[:], cnt_sb[:], float(CAP), None, op0=Alu.is_equal)
        nc.vector.select(ans[:], eqm[:], Tp[:], ans[:])
        nc.vector.select(clo[:], dmask[:], cnt_sb[:], clo[:])
        nc.vector.select(chi[:], dnmask[:], cnt_sb[:], chi[:])
        nc.vector.select(lo[:], dmask[:], Tp[:], lo[:])
        nc.vector.select(hi[:], dnmask[:], Tp[:], hi[:])

    # fallback if count==CAP was never hit (ties): use lo (count >= CAP)
    nc.vector.tensor_scalar(eqm[:], ans[:], 0.0, None, op0=Alu.is_le)
    nc.vector.select(ans[:], eqm[:], lo[:], ans[:])

    nc.sync.dma_start(dbg_out.ap(), dbg[:])
    thr_pad = misc.tile([EL, 16], F32, tag="thrpad")
    nc.vector.tensor_copy(thr_pad[:], ans[:].to_broadcast([EL, 16]))
    nc.sync.dma_start(thr_loc.ap(), thr_pad[:])

    nc.gpsimd.collective_compute(
        "AllGather",
        Alu.bypass,
        replica_groups=[list(range(NCORES))],
        ins=[thr_loc.ap().opt()],
        outs=[thr_glob.ap().opt()],
    )

    # ---- Phase D: apply thresholds to local tokens ----
    thr_sb = misc.tile([E, 1], F32, tag="thrsb")
    nc.sync.dma_start(
        thr_sb[:], thr_glob.ap().rearrange("c e f -> (c e) f")[:, 0:1]
    )
    Tap = psS.tile([P, 1], F32, tag="small")
    nc.tensor.matmul(Tap[:], r_sb[:], thr_sb[:])  # Tap[p] = thr[p // 8]

    av = scp.tile([P, 512], F32)  # [e*8 + sc, 512]: expert e, token sc*512+f
    nc.sync.dma_start(av[:], sc_loc.ap().rearrange("e (sc f) -> (e sc) f", f=512))

    base = misc.tile([P, 512], F32, tag="base")
    nc.vector.tensor_scalar(base[:], av[:], e_sb[:], None, op0=Alu.add)
    # comb  = mask * (score + e + 1); comb2 = mask * (e + 1)
    comb = misc.tile([P, 512], F32, tag="comb")
    nc.vector.scalar_tensor_tensor(
        comb[:], av[:], Tap[:], base[:], op0=Alu.is_ge, op1=Alu.mult
    )
    comb2 = misc.tile([P, 512], F32, tag="comb2")
    nc.vector.scalar_tensor_tensor(
        comb2[:], av[:], Tap[:], e_sb[:].to_broadcast([P, 512]),
        op0=Alu.is_ge, op1=Alu.mult,
    )
    # max over experts: transpose 128-blocks so (e, sc) lands in the free
    # dim, then a strided max-reduce over e. Token id = sc*512 + tt*128 + f.
    rw_r = rw_out.ap().rearrange("(sc tt f) -> tt f sc", sc=8, tt=4, f=P)
    ei_r = ei_out.ap().rearrange("(sc tt f) -> tt f sc", sc=8, tt=4, f=P)
    for tt in range(4):
        cps2 = psA.tile([P, P], F32, tag="tps")
        nc.tensor.transpose(cps2[:], comb[:, tt * P:(tt + 1) * P], id_sb[:])
        cmax = misc.tile([P, 8], F32, tag="cmax")
        nc.vector.tensor_reduce(
            cmax[:], cps2[:].rearrange("f (e sc) -> f sc e", e=E), AX.X, Alu.max
        )
        cps3 = psA.tile([P, P], F32, tag="tps")
        nc.tensor.transpose(cps3[:], comb2[:, tt * P:(tt + 1) * P], id_sb[:])
        cmax2 = misc.tile([P, 8], F32, tag="cmax2")
        nc.vector.tensor_reduce(
            cmax2[:], cps3[:].rearrange("f (e sc) -> f sc e", e=E), AX.X, Alu.max
        )
        frac = misc.tile([P, 8], F32, tag="frac")
        nc.vector.tensor_sub(frac[:], cmax[:], cmax2[:])
        eif = misc.tile([P, 8], F32, tag="eif")
        nc.vector.tensor_scalar(eif[:], cmax2[:], 1.0, None, op0=Alu.subtract)
        nc.vector.tensor_scalar_max(eif[:], eif[:], 0.0)
        ei_i = misc.tile([P, 8], I32, tag="eii")
        nc.vector.tensor_copy(ei_i[:], eif[:])
        nc.sync.dma_start(rw_r[tt], frac[:])
        nc.sync.dma_start(ei_r[tt], ei_i[:])

    ctx.close()


def _make_consts():
    pidx = np.arange(P)
    gc = np.zeros((P, EL), np.float32)
    for j in range(EL):
        gc[(pidx // 64) == j, j] = 1.0
    hc = np.zeros((EL, P), np.float32)
    for j in range(EL):
        hc[j, (pidx // 64) == j] = 1.0
    rc = np.zeros((E, P), np.float32)
    for k in range(E):
        rc[k, (pidx // 8) == k] = 1.0
    ec = (pidx // 8 + 1).astype(np.float32).reshape(P, 1)
    return {
        "ident": np.eye(P, dtype=np.float32),
        "gcst": gc,
        "hcst": hc,
        "rcst": rc,
        "ecst": ec,
    }


_NC_CACHE = {}


def _get_program():
    if "nc" not in _NC_CACHE:
        _NC_CACHE["nc"] = _build_program()
    return _NC_CACHE["nc"]


def _run(x, W, trace=False, **kw):
    x = np.ascontiguousarray(np.asarray(x, dtype=np.float32))
    W = np.ascontiguousarray(np.asarray(W, dtype=np.float32))
    # prepack W^T into per-d-chunk [128, 16] blocks side by side:
    # wt[p, k*16 + e] = W[e, k*128 + p]
    wtp = np.ascontiguousarray(
        W.reshape(E, 8, P).transpose(2, 1, 0).reshape(P, 8 * E)
    )
    consts = _make_consts()
    in_maps = []
    for c in range(NCORES):
        m = {"xs": x[BL * c:BL * (c + 1)].reshape(SL, D), "wt": wtp}
        m.update(consts)
        in_maps.append(m)
    nc = _get_program()
    res = bass_utils.run_bass_kernel_spmd(
        nc, in_maps, core_ids=list(range(NCORES)), trace=trace, **kw
    )
    outs = res.results
    _NC_CACHE["dbg"] = [outs[c].get("dbg_out") for c in range(NCORES)]
    scores = np.concatenate(
        [outs[c]["scores_out"] for c in range(NCORES)], axis=0
    ).reshape(B, S, E)
    rw = np.concatenate([outs[c]["rw_out"] for c in range(NCORES)]).reshape(B, S, 1)
    ei = np.concatenate([outs[c]["ei_out"] for c in range(NCORES)]).reshape(B, S, 1)
    return (rw, ei.astype(np.int32), scores), res


def kernel(x, W):
    (rw, ei, scores), _ = _run(x, W, trace=False)
    return rw, ei, scores


if __name__ == "__main__":
    xs = np.random.randn(B, S, D).astype(np.float32)
    Ws = (np.random.randn(E, D) / np.sqrt(D)).astype(np.float32)
    out = kernel(xs, Ws)
    print([o.shape for o in out])


# revision 17
# speedup vs baseline: 1.0071x; 1.0071x over previous
"""Expert-choice router kernel for Trainium2 (8 NeuronCores, Bass/Tile).

Strategy (data-parallel over batch, expert-parallel for selection):
  Phase A (per core, 2 batches):  logits^T = W @ x^T via PE (x transposed
          on-chip with PE transpose-mode), fused exp on ACT during PSUM
          evacuation, softmax over the sequence axis in [E, S] layout.
  Phase B: AllGather of per-core scores^T (256 KB each) so every core can
          see all tokens for its 2 experts.
  Phase C (per core, 2 experts): exact per-expert threshold = value t with
          count(score >= t) == capacity, found by branchless bisection /
          regula-falsi iterations on DVE with PE matmuls doing the
          cross-partition count reduction + threshold broadcast.
  Phase D: tiny AllGather of the 16 thresholds; per-token apply phase
          computes argmax-selected expert + routing weight with a
          pack-into-one-float trick and partition-folded maxes.
"""

import os
import sys

for _p in ("/opt/trn_rl_repo", "/root/.axon_site/_ro/trn_rl_repo"):
    if os.path.isdir(_p) and _p not in sys.path:
        sys.path.insert(0, _p)

import numpy as np


def _install_ntff_hook_module():
    """Provide antenv.axon_hooks (missing in this image) via sys.modules so
    run_bass_kernel_spmd(trace=True) can drive NTFF profiling through
    libaxon_pjrt.so. No-op when already present."""
    import types
    import importlib

    try:
        importlib.import_module("antenv.axon_hooks")
        return
    except ImportError:
        pass
    import contextlib
    import ctypes

    mod = types.ModuleType("antenv.axon_hooks")
    _state = {"hook": None}

    def set_axon_ntff_profile_hook(hook):
        _state["hook"] = hook

    def _via_ctypes(so_path):
        lib = ctypes.CDLL(so_path)
        if not hasattr(lib, "axon_start_nrt_profile"):
            return None
        lib.axon_start_nrt_profile.argtypes = [
            ctypes.POINTER(ctypes.c_int64),
            ctypes.c_size_t,
        ]
        lib.axon_start_nrt_profile.restype = ctypes.c_int64
        lib.axon_stop_nrt_profile.argtypes = [ctypes.c_char_p]
        lib.axon_stop_nrt_profile.restype = ctypes.c_int64

        @contextlib.contextmanager
        def _hook(output_dir, device_ids):
            import jax

            jax.devices()
            if device_ids:
                ids = (ctypes.c_int64 * len(device_ids))(*device_ids)
                rc = lib.axon_start_nrt_profile(ids, len(device_ids))
            else:
                rc = lib.axon_start_nrt_profile(None, 0)
            if rc != 0:
                raise RuntimeError(f"axon_start_nrt_profile rc={rc}")
            try:
                yield
            finally:
                n = lib.axon_stop_nrt_profile(str(output_dir).encode())
                print(f"ntff profile: {n} file(s) -> {output_dir}", file=sys.stderr)

        return _hook

    def get_axon_ntff_profile_hook():
        if _state["hook"] is None and os.path.exists("/opt/axon/libaxon_pjrt.so"):
            _state["hook"] = _via_ctypes("/opt/axon/libaxon_pjrt.so")
        return _state["hook"]

    mod.set_axon_ntff_profile_hook = set_axon_ntff_profile_hook
    mod.get_axon_ntff_profile_hook = get_axon_ntff_profile_hook
    sys.modules["antenv.axon_hooks"] = mod


_install_ntff_hook_module()

import concourse.bass as bass
import concourse.bacc as bacc
import concourse.mybir as mybir
import concourse.tile as tile
from concourse import bass_utils

Alu = mybir.AluOpType
Act = mybir.ActivationFunctionType
F32 = mybir.dt.float32
I32 = mybir.dt.int32
AX = mybir.AxisListType

B, S, D, E = 16, 2048, 1024, 16
NCORES = 8
BL = B // NCORES            # batches per core
SL = BL * S                 # tokens per core (4096)
N = B * S
CAP = int(1.25 * N / E)     # 2560
EL = E // NCORES            # experts per core in selection phase
P = 128

# Bisection schedule: fixed probes first (bracket the threshold region),
# then alternating bisection / regula-falsi. Tuned by host simulation with
# safety margin; each iteration is branchless so extra iterations are no-ops
# once count(>=T) == CAP has been observed (sticky `ans`).
PROBES = [1.0 / 2048, 2.0 / 2048, 4.0 / 2048, 8.0 / 2048]
ITERS = 26


def _build_program():
    nc = bacc.Bacc(
        "TRN2",
        target_bir_lowering=False,
        debug=False,
        num_devices=NCORES,
    )

    xs = nc.dram_tensor("xs", [SL, D], F32, kind="ExternalInput")
    wt = nc.dram_tensor("wt", [P, 8 * E], F32, kind="ExternalInput")
    ident = nc.dram_tensor("ident", [P, P], F32, kind="ExternalInput")
    gcst = nc.dram_tensor("gcst", [P, EL], F32, kind="ExternalInput")
    hcst = nc.dram_tensor("hcst", [EL, P], F32, kind="ExternalInput")
    rcst = nc.dram_tensor("rcst", [E, P], F32, kind="ExternalInput")
    ecst = nc.dram_tensor("ecst", [P, 1], F32, kind="ExternalInput")

    scores_out = nc.dram_tensor("scores_out", [SL, E], F32, kind="ExternalOutput")
    dbg_out = nc.dram_tensor("dbg_out", [EL, 64], F32, kind="ExternalOutput")
    rw_out = nc.dram_tensor("rw_out", [SL], F32, kind="ExternalOutput")
    ei_out = nc.dram_tensor("ei_out", [SL], I32, kind="ExternalOutput")

    sc_loc = nc.dram_tensor("sc_loc", [E, SL], F32, kind="Internal")
    sc_glob = nc.dram_tensor(
        "sc_glob", [NCORES, E, SL], F32, kind="Internal", addr_space="Shared"
    )
    thr_loc = nc.dram_tensor("thr_loc", [EL, 16], F32, kind="Internal")
    thr_glob = nc.dram_tensor(
        "thr_glob", [NCORES, EL, 16], F32, kind="Internal", addr_space="Shared"
    )

    with tile.TileContext(nc) as tc:
        _build_tile(tc, nc, locals())

    nc.compile()
    return nc


def _build_tile(tc, nc, t):
    xs, wt, ident = t["xs"], t["wt"], t["ident"]
    gcst, hcst, rcst, ecst = t["gcst"], t["hcst"], t["rcst"], t["ecst"]
    scores_out, rw_out, ei_out = t["scores_out"], t["rw_out"], t["ei_out"]
    dbg_out = t["dbg_out"]
    sc_loc, sc_glob, thr_loc, thr_glob = (
        t["sc_loc"], t["sc_glob"], t["thr_loc"], t["thr_glob"],
    )

    from contextlib import ExitStack

    ctx = ExitStack()
    consts = ctx.enter_context(tc.tile_pool(name="consts", bufs=1))
    xin = ctx.enter_context(tc.tile_pool(name="xin", bufs=3))
    xtp = ctx.enter_context(tc.tile_pool(name="xtp", bufs=3))
    scp = ctx.enter_context(tc.tile_pool(name="scp", bufs=1))
    misc = ctx.enter_context(tc.tile_pool(name="misc", bufs=2))
    psA = ctx.enter_context(tc.tile_pool(name="psA", bufs=2, space="PSUM"))
    psL = ctx.enter_context(tc.tile_pool(name="psL", bufs=2, space="PSUM"))
    psS = ctx.enter_context(tc.tile_pool(name="psS", bufs=2, space="PSUM"))

    # ---- constants ----
    wt_sb = consts.tile([P, 8 * E], F32)
    nc.sync.dma_start(wt_sb[:], wt.ap())
    id_sb = consts.tile([P, P], F32)
    nc.sync.dma_start(id_sb[:], ident.ap())
    g_sb = consts.tile([P, EL], F32)
    nc.sync.dma_start(g_sb[:], gcst.ap())
    h_sb = consts.tile([EL, P], F32)
    nc.sync.dma_start(h_sb[:], hcst.ap())
    r_sb = consts.tile([E, P], F32)
    nc.sync.dma_start(r_sb[:], rcst.ap())
    e_sb = consts.tile([P, 1], F32)
    nc.sync.dma_start(e_sb[:], ecst.ap())
    ones_sb = consts.tile([P, 512], F32)
    nc.vector.memset(ones_sb[:], 1.0)

    # ---- Phase A: logits^T + exp, per 512-token group ----
    scT = scp.tile([E, SL], F32)          # exp(logits)^T, later scores^T
    denp = consts.tile([E, 8], F32)       # per-group exp-sum partials
    xs_r = xs.ap().rearrange("(g t p) d -> g p t d", p=P, t=4)

    scopeA = nc.named_scope("phaseA")
    scopeA.__enter__()
    for g in range(8):
        x_sb = xin.tile([P, 4, D], F32)
        nc.sync.dma_start(x_sb[:], xs_r[g])
        lps = psL.tile([E, 512], F32)
        for k in range(8):
            tps = psA.tile([P, 512], F32, tag="tps")
            for tt in range(4):
                nc.tensor.transpose(
                    tps[:, tt * P:(tt + 1) * P],
                    x_sb[:, tt, k * P:(k + 1) * P],
                    id_sb[:],
                )
            xt_sb = xtp.tile([P, 512], F32, tag="xt")
            nc.vector.tensor_copy(xt_sb[:], tps[:])
            nc.tensor.matmul(
                lps[:],
                wt_sb[:, k * E:(k + 1) * E],
                xt_sb[:],
                start=(k == 0),
                stop=(k == 7),
            )
        # fused exp during PSUM evacuation + free row-sum partial
        nc.scalar.activation(
            scT[:, g * 512:(g + 1) * 512],
            lps[:],
            Act.Exp,
            accum_out=denp[:, g:g + 1],
        )

    # ---- softmax normalization over sequence axis (per local batch) ----
    den = misc.tile([E, BL], F32, tag="den")
    rden = misc.tile([E, BL], F32, tag="rden")
    for b in range(BL):
        nc.vector.tensor_reduce(
            den[:, b:b + 1], denp[:, b * 4:(b + 1) * 4], AX.X, Alu.add
        )
    nc.vector.reciprocal(rden[:], den[:])
    for b in range(BL):
        nc.vector.tensor_scalar(
            scT[:, b * S:(b + 1) * S],
            scT[:, b * S:(b + 1) * S],
            rden[:, b:b + 1],
            None,
            op0=Alu.mult,
        )

    # scores^T -> DRAM (collective input + apply-phase reload source)
    nc.sync.dma_start(sc_loc.ap(), scT[:])

    # ---- scores output in [s, e] orientation via PE transpose ----
    for tt in range(SL // P):
        sps = psS.tile([P, E], F32, tag="small")
        nc.tensor.transpose(sps[:], scT[:, tt * P:(tt + 1) * P], id_sb[:E, :E])
        so_sb = xtp.tile([P, E], F32, tag="so")
        nc.vector.tensor_copy(so_sb[:], sps[:])
        nc.sync.dma_start(scores_out.ap()[tt * P:(tt + 1) * P, :], so_sb[:])

    scopeA.__exit__(None, None, None)
    # ---- Phase B: exchange scores ----
    scopeB = nc.named_scope("gather")
    scopeB.__enter__()
    nc.gpsimd.collective_compute(
        "AllGather",
        Alu.bypass,
        replica_groups=[list(range(NCORES))],
        ins=[sc_loc.ap().opt()],
        outs=[sc_glob.ap().opt()],
    )

    # ---- Phase C: exact per-expert thresholds (2 experts per core) ----
    pid = nc.partition_id()
    v_sb = scp.tile([P, 512], F32)        # [2 experts x 64 chunks, 512]
    sgl = sc_glob.ap()                     # [8, 16, 4096]
    for ei in range(EL):
        src = sgl[:, bass.ds(EL * pid + ei, 1), :].rearrange(
            "c o (u f) -> c (o u) f", f=512
        )
        nc.sync.dma_start(v_sb[ei * 64:(ei + 1) * 64, :], src)

    scopeB.__exit__(None, None, None)
    scopeC = nc.named_scope("bisect")
    scopeC.__enter__()
    lo = misc.tile([EL, 1], F32, tag="lo")
    hi = misc.tile([EL, 1], F32, tag="hi")
    clo = misc.tile([EL, 1], F32, tag="clo")
    chi = misc.tile([EL, 1], F32, tag="chi")
    ans = misc.tile([EL, 1], F32, tag="ans")
    Tp = misc.tile([EL, 1], F32, tag="Tp")
    U32 = mybir.dt.uint32
    dmask = misc.tile([EL, 1], U32, tag="dmask")
    dnmask = misc.tile([EL, 1], U32, tag="dnmask")
    eqm = misc.tile([EL, 1], U32, tag="eqm")
    sc1 = misc.tile([EL, 1], F32, tag="sc1")
    sc2 = misc.tile([EL, 1], F32, tag="sc2")
    sc3 = misc.tile([EL, 1], F32, tag="sc3")
    cnt_sb = misc.tile([EL, 1], F32, tag="cntsb")
    nc.vector.memset(lo[:], 0.0)
    nc.vector.memset(hi[:], 1.0)
    nc.vector.memset(clo[:], float(N))
    nc.vector.memset(chi[:], 0.0)
    nc.vector.memset(ans[:], 0.0)

    cpp = misc.tile([P, 1], F32, tag="cpp")
    scr = misc.tile([P, 512], F32, tag="scr")
    dbg = misc.tile([EL, 64], F32, tag="dbg")
    nc.vector.memset(dbg[:], 0.0)

    for it in range(ITERS):
        if it < len(PROBES):
            nc.vector.memset(Tp[:], float(PROBES[it]))
        elif it % 2 == 0:
            # bisection step: T = (lo + hi) / 2
            nc.vector.tensor_add(Tp[:], lo[:], hi[:])
            nc.vector.tensor_scalar_mul(Tp[:], Tp[:], 0.5)
        else:
            # regula falsi: T = lo + (clo-CAP)/(clo-chi) * (hi-lo)
            nc.vector.tensor_scalar(sc1[:], clo[:], float(CAP), None, op0=Alu.subtract)
            nc.vector.tensor_sub(sc2[:], clo[:], chi[:])
            nc.vector.reciprocal(sc3[:], sc2[:])
            nc.vector.tensor_mul(sc1[:], sc1[:], sc3[:])
            nc.vector.tensor_sub(sc2[:], hi[:], lo[:])
            nc.vector.tensor_mul(sc1[:], sc1[:], sc2[:])
            nc.vector.tensor_add(Tp[:], lo[:], sc1[:])

        # broadcast T to the 128 partitions: Tb[p] = Tp[p // 64]
        Tb = psS.tile([P, 1], F32, tag="small")
        nc.tensor.matmul(Tb[:], h_sb[:], Tp[:])
        # per-partition counts of (v >= T)
        for ei in range(EL):
            sl = slice(ei * 64, (ei + 1) * 64)
            nc.vector.scalar_tensor_tensor(
                scr[sl, :],
                v_sb[sl, :],
                Tb[sl, :],
                ones_sb[sl, :],
                op0=Alu.is_ge,
                op1=Alu.mult,
                accum_out=cpp[sl, :],
            )
        # total count per expert: cps[e] = sum over the expert's 64 partitions
        cps = psS.tile([EL, 1], F32, tag="small2")
        nc.tensor.matmul(cps[:], g_sb[:], cpp[:])

        nc.vector.tensor_copy(cnt_sb[:], cps[:])
        nc.vector.tensor_copy(dbg[:, it:it + 1], Tp[:])
        nc.vector.tensor_copy(dbg[:, 32 + it:33 + it], cnt_sb[:])
        # branchless bracket update; out must only alias on_false (select
        # lowers to copy(on_false) + copy_predicated(on_true))
        nc.vector.tensor_scalar(dmask[:], cnt_sb[:], float(CAP), None, op0=Alu.is_ge)
        nc.vector.tensor_scalar(dnmask[:], cnt_sb[:], float(CAP), None, op0=Alu.is_lt)
        nc.vector.tensor_scalar(eqm[:], cnt_sb[:], float(CAP), None, op0=Alu.is_equal)
        nc.vector.select(ans[:], eqm[:], Tp[:], ans[:])
        nc.vector.select(clo[:], dmask[:], cnt_sb[:], clo[:])
        nc.vector.select(chi[:], dnmask[:], cnt_sb[:], chi[:])
        nc.vector.select(lo[:], dmask[:], Tp[:], lo[:])
        nc.vector.select(hi[:], dnmask[:], Tp[:], hi[:])

    # fallback if count==CAP was never hit (ties): use lo (count >= CAP)
    nc.vector.tensor_scalar(eqm[:], ans[:], 0.0, None, op0=Alu.is_le)
    nc.vector.select(ans[:], eqm[:], lo[:], ans[:])

    nc.sync.dma_start(dbg_out.ap(), dbg[:])
    thr_pad = misc.tile([EL, 16], F32, tag="thrpad")
    nc.vector.tensor_copy(thr_pad[:], ans[:].to_broadcast([EL, 16]))
    nc.sync.dma_start(thr_loc.ap(), thr_pad[:])

    nc.gpsimd.collective_compute(
        "AllGather",
        Alu.bypass,
        replica_groups=[list(range(NCORES))],
        ins=[thr_loc.ap().opt()],
        outs=[thr_glob.ap().opt()],
    )

    scopeC.__exit__(None, None, None)
    scopeD = nc.named_scope("apply")
    scopeD.__enter__()
    # ---- Phase D: apply thresholds to local tokens ----
    thr_sb = misc.tile([E, 1], F32, tag="thrsb")
    nc.sync.dma_start(
        thr_sb[:], thr_glob.ap().rearrange("c e f -> (c e) f")[:, 0:1]
    )
    Tap = psS.tile([P, 1], F32, tag="small")
    nc.tensor.matmul(Tap[:], r_sb[:], thr_sb[:])  # Tap[p] = thr[p // 8]

    av = scp.tile([P, 512], F32)  # [e*8 + sc, 512]: expert e, token sc*512+f
    nc.sync.dma_start(av[:], sc_loc.ap().rearrange("e (sc f) -> (e sc) f", f=512))

    # comb2 = mask * (e + 1)
    comb2 = misc.tile([P, 512], F32, tag="comb2")
    nc.vector.scalar_tensor_tensor(
        comb2[:], av[:], Tap[:], e_sb[:].to_broadcast([P, 512]),
        op0=Alu.is_ge, op1=Alu.mult,
    )
    # max over experts: transpose 128-blocks so (e, sc) lands in the free
    # dim, then a strided max-reduce over e. Token id = sc*512 + tt*128 + f.
    rw_r = rw_out.ap().rearrange("(sc tt f) -> tt f sc", sc=8, tt=4, f=P)
    ei_r = ei_out.ap().rearrange("(sc tt f) -> tt f sc", sc=8, tt=4, f=P)
    for tt in range(4):
        cps3 = psA.tile([P, P], F32, tag="tps")
        nc.tensor.transpose(cps3[:], comb2[:, tt * P:(tt + 1) * P], id_sb[:])
        avt = psA.tile([P, P], F32, tag="tps")
        nc.tensor.transpose(avt[:], av[:, tt * P:(tt + 1) * P], id_sb[:])
        cmax2 = misc.tile([P, 8], F32, tag="cmax2")
        nc.vector.tensor_reduce(
            cmax2[:], cps3[:].rearrange("f (e sc) -> f sc e", e=E), AX.X, Alu.max
        )
        # indicator of the winning expert slot, then gather its exact score
        ind = misc.tile([P, P], F32, tag="ind")
        nc.vector.tensor_tensor(
            ind[:].rearrange("f (e sc) -> f e sc", e=E),
            cps3[:].rearrange("f (e sc) -> f e sc", e=E),
            cmax2[:, None, :].to_broadcast([P, E, 8]),
            Alu.is_equal,
        )
        nc.vector.tensor_tensor(ind[:], ind[:], avt[:], Alu.mult)
        frac = misc.tile([P, 8], F32, tag="frac")
        nc.vector.tensor_reduce(
            frac[:], ind[:].rearrange("f (e sc) -> f sc e", e=E), AX.X, Alu.max
        )
        selok = misc.tile([P, 8], F32, tag="selok")
        nc.vector.tensor_scalar(selok[:], cmax2[:], 0.0, None, op0=Alu.is_gt)
        nc.vector.tensor_mul(frac[:], frac[:], selok[:])
        eif = misc.tile([P, 8], F32, tag="eif")
        nc.vector.tensor_scalar(eif[:], cmax2[:], 1.0, None, op0=Alu.subtract)
        nc.vector.tensor_scalar_max(eif[:], eif[:], 0.0)
        ei_i = misc.tile([P, 8], I32, tag="eii")
        nc.vector.tensor_copy(ei_i[:], eif[:])
        nc.sync.dma_start(rw_r[tt], frac[:])
        nc.sync.dma_start(ei_r[tt], ei_i[:])

    scopeD.__exit__(None, None, None)
    ctx.close()


def _make_consts():
    pidx = np.arange(P)
    gc = np.zeros((P, EL), np.float32)
    for j in range(EL):
        gc[(pidx // 64) == j, j] = 1.0
    hc = np.zeros((EL, P), np.float32)
    for j in range(EL):
        hc[j, (pidx // 64) == j] = 1.0
    rc = np.zeros((E, P), np.float32)
    for k in range(E):
        rc[k, (pidx // 8) == k] = 1.0
    ec = (pidx // 8 + 1).astype(np.float32).reshape(P, 1)
    return {
        "ident": np.eye(P, dtype=np.float32),
        "gcst": gc,
        "hcst": hc,
        "rcst": rc,
        "ecst": ec,
    }


_NC_CACHE = {}


def _get_program():
    if "nc" not in _NC_CACHE:
        _NC_CACHE["nc"] = _build_program()
    return _NC_CACHE["nc"]


def _run(x, W, trace=False, **kw):
    x = np.ascontiguousarray(np.asarray(x, dtype=np.float32))
    W = np.ascontiguousarray(np.asarray(W, dtype=np.float32))
    # prepack W^T into per-d-chunk [128, 16] blocks side by side:
    # wt[p, k*16 + e] = W[e, k*128 + p]
    wtp = np.ascontiguousarray(
        W.reshape(E, 8, P).transpose(2, 1, 0).reshape(P, 8 * E)
    )
    consts = _make_consts()
    in_maps = []
    for c in range(NCORES):
        m = {"xs": x[BL * c:BL * (c + 1)].reshape(SL, D), "wt": wtp}
        m.update(consts)
        in_maps.append(m)
    nc = _get_program()
    res = bass_utils.run_bass_kernel_spmd(
        nc, in_maps, core_ids=list(range(NCORES)), trace=trace, **kw
    )
    outs = res.results
    _NC_CACHE["dbg"] = [outs[c].get("dbg_out") for c in range(NCORES)]
    scores = np.concatenate(
        [outs[c]["scores_out"] for c in range(NCORES)], axis=0
    ).reshape(B, S, E)
    rw = np.concatenate([outs[c]["rw_out"] for c in range(NCORES)]).reshape(B, S, 1)
    ei = np.concatenate([outs[c]["ei_out"] for c in range(NCORES)]).reshape(B, S, 1)
    return (rw, ei.astype(np.int32), scores), res


def kernel(x, W):
    (rw, ei, scores), _ = _run(x, W, trace=False)
    return rw, ei, scores


if __name__ == "__main__":
    xs = np.random.randn(B, S, D).astype(np.float32)
    Ws = (np.random.randn(E, D) / np.sqrt(D)).astype(np.float32)
    out = kernel(xs, Ws)
    print([o.shape for o in out])
